# revision 4
# baseline (speedup 1.0000x reference)
"""Trainium2 Bass kernel for nn_DS_EmbeddingGenerator (4-layer GCN + inter-layer
message passing + MLP head), 8-core SPMD.

Strategy (graph/data parallel per sharding hint):
- Nodes partitioned in 8 contiguous dst-slices of 25000 (padded to 25088 = 196
  groups of 128). Each core aggregates the edges whose dst lands in its slice.
- GCN factorization: conv(h,W,b) = inv[dst] * (segsum_{edges+self}((inv*h)[src])) @ W + b
  so no per-edge coefficients are needed; self-loops are appended to the edge list.
- Per-edge gather of table rows via dma_gather (int16 idx, 7 static 32768-row
  src windows; 128B payload on 256B-stride tables via a patched elem_size).
- Segment-sum on chip: f32 one-hot (is_equal vs iota) + PE matmul accumulating
  into PSUM per 128-dst group; epilogue applies inv scaling, weights, bias, relu.
- Full tables (h1*inv per layer; last_i) are exchanged with AllGather collectives.
- Final MLP fused per group; output written per-core slice, concatenated on host.
"""
import inspect
import textwrap
import warnings

import numpy as np

warnings.filterwarnings("ignore")

L, N, E = 4, 200000, 3200000
F, H, D, LH = 20, 32, 32, 64
NC = 8
NPC = 25000            # real nodes per core
P = 128
GPC = 196              # groups per core (196*128 = 25088)
NPAD = GPC * P         # padded nodes per core
WIN = 32768
TABROWS = NC * NPAD    # AG table rows (200704)
NWIN = (TABROWS + WIN - 1) // WIN  # 7
NB = 6                 # groups per gather batch

_f32 = np.float32


def _patch_gather():
    import concourse.bass as bass

    if hasattr(bass.BassGpSimd, "dma_gather_unrestricted"):
        return
    src = inspect.getsource(bass.BassGpSimd.dma_gather)
    src = src.replace(
        "assert (\n            elem_size_bytes > 0 and elem_size_bytes % 256 == 0\n        )  # transpose restriction",
        "assert elem_size_bytes > 0",
    )
    ns = vars(bass).copy()
    exec(textwrap.dedent(src), ns)
    bass.BassGpSimd.dma_gather_unrestricted = ns["dma_gather"]


# ---------------------------------------------------------------- host preprocessing

def _prep_round(dst, src, tabidx):
    """Shard one aggregation round across cores.

    dst: global dst (int64) in [0, N); src unused except via tabidx;
    tabidx: table row for each edge (int64) in [0, table rows).
    Returns (T, per-core dicts with gidx(int16 wrapped+replicated), dstf(f32)).
    T[g][w] tile table uniform across cores.
    """
    core = dst // NPC
    dloc = dst - core * NPC
    grp = dloc >> 7
    gloc = dloc & 127
    w = tabidx >> 15
    lidx = tabidx - (w << 15)

    cnt = np.zeros((NC, GPC, NWIN), np.int64)
    key_cgw = (core * GPC + grp) * NWIN + w
    np.add.at(cnt.reshape(-1), key_cgw, 1)
    tiles = -(-cnt // P)          # ceil
    T = tiles.max(axis=0)         # [GPC, NWIN]
    # safety: every (g,w) cell capacity
    total_tiles = int(T.sum())

    # slot layout per core: order (g, w, tile, pos); per (c,g,w) edges then pads
    # build with a global sort by (core, g, w, lidx)
    order = np.lexsort((lidx, w, grp, core))
    core_s = core[order]
    grp_s = grp[order]
    w_s = w[order]
    lidx_s = lidx[order]
    gloc_s = gloc[order]

    # slot base for each (g, w): offset in tiles
    cell_tile_off = np.zeros((GPC, NWIN), np.int64)
    flat_T = T.reshape(-1)
    cell_tile_off.reshape(-1)[1:] = np.cumsum(flat_T)[:-1]

    # rank of each edge within its (c,g,w) cell
    k = (core_s * GPC + grp_s) * NWIN + w_s
    is_new = np.empty(len(k), bool)
    is_new[0] = True
    is_new[1:] = k[1:] != k[:-1]
    seg_start = np.where(is_new)[0]
    rank = np.arange(len(k)) - np.repeat(seg_start, np.diff(np.append(seg_start, len(k))))

    slot = cell_tile_off.reshape(-1)[grp_s * NWIN + w_s] * P + rank

    per_core = []
    nslots = total_tiles * P
    for c in range(NC):
        m = core_s == c
        gi = np.zeros(nslots, np.int16)          # pad idx 0 (valid, zeroed by onehot)
        df = np.full(nslots, -1.0, _f32)          # pad dst -> one-hot all-zero
        gi[slot[m]] = lidx_s[m].astype(np.int16)
        df[slot[m]] = gloc_s[m].astype(_f32)
        per_core.append((gi, df))

    # gather-call structure: per (batch of NB groups, w): tiles
    batches = [(b0, min(b0 + NB, GPC)) for b0 in range(0, GPC, NB)]
    calls = []                     # (w, tile_start, ntiles) in slot space? need per-(g,w) runs
    # slot space is ordered g-major then w: a (batch,w) call covers non-contiguous
    # per-g runs -> instead make calls per (g,w) contiguous? Reorganize: we emit
    # calls per (batch, w) as the union of per-g cell ranges which ARE contiguous
    # in slot space only per (g,w). To keep one DMA per call, reorder slots:
    # order (w-major within batch). Simpler: calls per (g, w) are contiguous; we
    # bundle consecutive groups' (g,w) cells by issuing per (batch, w) a single
    # call over a REARRANGED index array built here: concat cells (g in batch).
    # We therefore build a separate call-ordered gidx array.
    call_specs = []                # (ntiles,) per call in emission order
    gidx_call_parts = [[] for _ in range(NC)]
    for (b0, b1) in batches:
        for wq in range(NWIN):
            nt = int(T[b0:b1, wq].sum())
            call_specs.append((wq, nt))
            if nt == 0:
                continue
            for c in range(NC):
                gi = per_core[c][0]
                parts = []
                for g in range(b0, b1):
                    t0 = cell_tile_off[g, wq] * P
                    parts.append(gi[t0 : t0 + T[g, wq] * P])
                gidx_call_parts[c].append(np.concatenate(parts))

    out_cores = []
    for c in range(NC):
        flat = (
            np.concatenate(gidx_call_parts[c])
            if gidx_call_parts[c]
            else np.zeros(0, np.int16)
        )
        # wrap 16 + replicate to 128 partitions
        wrapped = np.ascontiguousarray(flat.reshape(-1, 16).T)
        rep = np.tile(wrapped, (8, 1))
        # dstf in (g, w, t) order == slot order
        out_cores.append({"gidx": rep, "dstf": per_core[c][1].reshape(-1, P).T.copy()})
    return T, call_specs, batches, out_cores


def _self_loops():
    n = np.arange(N, dtype=np.int64)
    return n


def preprocess(xs, edge_index, layer_edge_index, deg_scalars,
               gcn_w1, gcn_b1, gcn_w2, gcn_b2, lin1_w, lin1_b, lin2_w, lin2_b):
    xs = np.asarray(xs, _f32)
    edge_index = np.asarray(edge_index, np.int64)
    layer_edge_index = np.asarray(layer_edge_index, np.int64)

    n = _self_loops()
    tr = lambda v: NPAD * (v // NPC) + (v % NPC)   # node id -> AG-table row

    rounds = []        # list of (T, call_specs, batches, per_core_arrays)
    # conv1/conv2 per layer
    for ell in range(L):
        d = edge_index[ell, 0]
        s = edge_index[ell, 1]
        dall = np.concatenate([d, n])
        sall = np.concatenate([s, n])
        rounds.append(_prep_round(dall, sall, sall))            # conv1: g1 natural rows
        rounds.append(_prep_round(dall, sall, tr(sall)))        # conv2: AG table rows
    for i in range(1, L):
        recv = layer_edge_index[i, 0]
        s = layer_edge_index[i, 1]
        rounds.append(_prep_round(recv, s, tr(s)))              # msg_i

    # counts for inv (in-degree per layer, global & per-core local)
    G_ALL = 1563  # ceil(N/128) with N padded to 200064
    NPADALL = G_ALL * P
    cnt_glob = np.zeros((L, NPADALL), _f32)
    for ell in range(L):
        cnt_glob[ell, :N] = np.bincount(edge_index[ell, 0], minlength=N).astype(_f32)
    cnt_glob_t = cnt_glob.reshape(L, G_ALL, P).transpose(0, 2, 1).copy()  # [L,128,1563]

    cnt_loc = np.zeros((NC, L, NPAD), _f32)
    for c in range(NC):
        cnt_loc[c, :, :NPC] = cnt_glob[:, c * NPC : (c + 1) * NPC]
    cnt_loc_t = cnt_loc.reshape(NC, L, GPC, P).transpose(0, 1, 3, 2).copy()  # [c][L,128,196]

    # xs tiled [L, 128, 1563*F]
    xs_pad = np.zeros((L, NPADALL, F), _f32)
    xs_pad[:, :N] = xs
    xs_t = xs_pad.reshape(L, G_ALL, P, F).transpose(0, 2, 1, 3).reshape(L, P, G_ALL * F).copy()

    consts = {
        "xs_t": xs_t,
        "cnt_glob": cnt_glob_t,
        "w1": np.asarray(gcn_w1, _f32),                     # [L,20,32]
        "b1_rep": np.tile(np.asarray(gcn_b1, _f32)[:, None, :], (1, P, 1)),  # [L,128,32]
        "w2": np.asarray(gcn_w2, _f32),                     # [L,32,32]
        "b2_rep": np.tile(np.asarray(gcn_b2, _f32)[:, None, :], (1, P, 1)),
        "lin1_w": np.asarray(lin1_w, _f32),                 # [32,64]
        "lin1_b_col": np.asarray(lin1_b, _f32)[:, None],    # [64,1]
        "lin2_w": np.asarray(lin2_w, _f32),                 # [64,32]
        "lin2_b_rep": np.tile(np.asarray(lin2_b, _f32)[None, :], (P, 1)),  # [128,32]
        "deg_rep": np.tile(np.asarray(deg_scalars, _f32)[:, None, None], (1, P, D)),  # [L,128,32]
    }
    return rounds, consts, cnt_loc_t


# ---------------------------------------------------------------- device program

def build_program(rounds, consts):
    import concourse.bass as bass
    import concourse.mybir as mybir
    import concourse.tile as tile
    from concourse import bacc
    from concourse.masks import make_identity

    _patch_gather()
    G_ALL = 1563

    nc = bacc.Bacc("TRN2", target_bir_lowering=False, debug=False, num_devices=NC)
    dt = mybir.dt

    # ---- I/O ----
    xs_t = nc.dram_tensor("xs_t", [L, P, G_ALL * F], dt.float32, kind="ExternalInput")
    cnt_glob = nc.dram_tensor("cnt_glob", [L, P, G_ALL], dt.float32, kind="ExternalInput")
    cnt_loc = nc.dram_tensor("cnt_loc", [L, P, GPC], dt.float32, kind="ExternalInput")
    w1_in = nc.dram_tensor("w1", [L, F, H], dt.float32, kind="ExternalInput")
    b1_in = nc.dram_tensor("b1_rep", [L, P, H], dt.float32, kind="ExternalInput")
    w2_in = nc.dram_tensor("w2", [L, H, D], dt.float32, kind="ExternalInput")
    b2_in = nc.dram_tensor("b2_rep", [L, P, D], dt.float32, kind="ExternalInput")
    l1w_in = nc.dram_tensor("lin1_w", [D, LH], dt.float32, kind="ExternalInput")
    l1b_in = nc.dram_tensor("lin1_b_col", [LH, 1], dt.float32, kind="ExternalInput")
    l2w_in = nc.dram_tensor("lin2_w", [LH, D], dt.float32, kind="ExternalInput")
    l2b_in = nc.dram_tensor("lin2_b_rep", [P, D], dt.float32, kind="ExternalInput")
    deg_in = nc.dram_tensor("deg_rep", [L, P, D], dt.float32, kind="ExternalInput")

    g_ins, d_ins = [], []
    for r, (T, call_specs, batches, per_core) in enumerate(rounds):
        gshape = per_core[0]["gidx"].shape
        dshape = per_core[0]["dstf"].shape
        g_ins.append(nc.dram_tensor(f"gidx{r}", list(gshape), dt.int16, kind="ExternalInput"))
        d_ins.append(nc.dram_tensor(f"dstf{r}", list(dshape), dt.float32, kind="ExternalInput"))

    out = nc.dram_tensor("out", [NPC, D], dt.float32, kind="ExternalOutput")

    # internal DRAM
    g1tab = [nc.dram_tensor(f"g1tab{l}", [G_ALL * P, 64], dt.float32) for l in range(L)]
    feat_dram = nc.dram_tensor("feat_dram", [NPAD, D], dt.float32)
    last3_dram = nc.dram_tensor("last3_dram", [NPAD, D], dt.float32)
    ag_in = [nc.dram_tensor(f"agin{i}", [NPAD, 64], dt.float32) for i in range(7)]
    ag_out = [
        nc.dram_tensor(f"agout{i}", [TABROWS, 64], dt.float32, addr_space="Shared")
        for i in range(7)
    ]
    # AG index: 0..3 = g2 tables layers 0..3 ; 4..6 = last_0..last_2

    core_ids = list(range(NC))

    with tile.TileContext(nc) as tc:
        with (
            tc.tile_pool(name="const", bufs=1) as cpool,
            tc.tile_pool(name="g1p", bufs=3) as g1pool,
            tc.tile_pool(name="gather", bufs=2) as gpool,
            tc.tile_pool(name="meta", bufs=3) as mpool,
            tc.tile_pool(name="oh", bufs=2) as ohpool,
            tc.tile_pool(name="epi", bufs=4) as epool,
            tc.tile_pool(name="prol", bufs=2) as ppool,
            tc.tile_pool(name="psA", bufs=2, space="PSUM") as psA,
            tc.tile_pool(name="psB", bufs=2, space="PSUM") as psB,
            tc.tile_pool(name="psC", bufs=2, space="PSUM") as psC,
        ):
            # ---------- constants ----------
            iota_i = cpool.tile([P, P], dt.int32)
            nc.gpsimd.iota(iota_i[:], pattern=[[1, P]], base=0, channel_multiplier=0)
            iota_f = cpool.tile([P, P], dt.float32)
            nc.vector.tensor_copy(iota_f[:], iota_i[:])
            ident = cpool.tile([P, P], dt.float32)
            make_identity(nc, ident[:])
            ones_col = cpool.tile([P, 1], dt.float32)
            nc.gpsimd.memset(ones_col[:], 1.0)

            w1_t = cpool.tile([F, L * H], dt.float32)
            for l in range(L):
                nc.sync.dma_start(w1_t[:, l * H : (l + 1) * H], w1_in[l])
            w2_t = cpool.tile([H, L * D], dt.float32)
            for l in range(L):
                nc.sync.dma_start(w2_t[:, l * D : (l + 1) * D], w2_in[l])
            b1_t = cpool.tile([P, L * H], dt.float32)
            for l in range(L):
                nc.sync.dma_start(b1_t[:, l * H : (l + 1) * H], b1_in[l])
            b2_t = cpool.tile([P, L * D], dt.float32)
            for l in range(L):
                nc.sync.dma_start(b2_t[:, l * D : (l + 1) * D], b2_in[l])
            l1w_t = cpool.tile([D, LH], dt.float32)
            nc.sync.dma_start(l1w_t[:], l1w_in[:])
            l1b_t = cpool.tile([LH, 1], dt.float32)
            nc.sync.dma_start(l1b_t[:], l1b_in[:])
            l2w_t = cpool.tile([LH, D], dt.float32)
            nc.sync.dma_start(l2w_t[:], l2w_in[:])
            l2b_t = cpool.tile([P, D], dt.float32)
            nc.sync.dma_start(l2b_t[:], l2b_in[:])
            degr = cpool.tile([P, L * D], dt.float32)
            for l in range(L):
                nc.sync.dma_start(degr[:, l * D : (l + 1) * D], deg_in[l])
            invdeg = cpool.tile([P, L * D], dt.float32)
            nc.vector.reciprocal(invdeg[:], degr[:])

            # inv tables
            inv_glob = cpool.tile([P, L * G_ALL], dt.float32)
            inv_loc = cpool.tile([P, L * GPC], dt.float32)
            for l in range(L):
                cg = ppool.tile([P, G_ALL], dt.float32, tag="cg")
                nc.sync.dma_start(cg[:], cnt_glob[l])
                nc.vector.tensor_scalar_add(cg[:], cg[:], 1.0)
                nc.vector.reciprocal(cg[:], cg[:])
                nc.scalar.activation(
                    inv_glob[:, l * G_ALL : (l + 1) * G_ALL], cg[:],
                    mybir.ActivationFunctionType.Sqrt,
                )
                cl = ppool.tile([P, GPC], dt.float32, tag="cl")
                nc.sync.dma_start(cl[:], cnt_loc[l])
                nc.vector.tensor_scalar_add(cl[:], cl[:], 1.0)
                nc.vector.reciprocal(cl[:], cl[:])
                nc.scalar.activation(
                    inv_loc[:, l * GPC : (l + 1) * GPC], cl[:],
                    mybir.ActivationFunctionType.Sqrt,
                )


            # ---------- g1 table builds ----------
            XB = 32
            for l in range(L):
                g1ap = g1tab[l][:].rearrange("(g p) e -> p g e", p=P)
                for G0 in range(0, G_ALL, XB):
                    B = min(XB, G_ALL - G0)
                    xt = g1pool.tile([P, XB * F], dt.float32, tag="xt")
                    nc.sync.dma_start(
                        xt[:, : B * F], xs_t[l][:, G0 * F : (G0 + B) * F]
                    )
                    sc = g1pool.tile([P, XB * F], dt.float32, tag="sc")
                    nc.vector.tensor_tensor(
                        out=sc[:, : B * F].rearrange("p (b f) -> p b f", b=B),
                        in0=xt[:, : B * F].rearrange("p (b f) -> p b f", b=B),
                        in1=inv_glob[:, l * G_ALL + G0 : l * G_ALL + G0 + B]
                        .rearrange("p (b o) -> p b o", o=1)
                        .to_broadcast([P, B, F]),
                        op=mybir.AluOpType.mult,
                    )
                    nc.sync.dma_start(
                        g1ap[:, G0 : G0 + B, :F],
                        sc[:, : B * F].rearrange("p (b f) -> p b f", b=B),
                    )

            # ---------- round runner ----------
            def run_round(r, table_ap, d_in, epilogue, row_major=False):
                T, call_specs, batches, _ = rounds[r]
                gidx = g_ins[r]
                dstf = d_ins[r]
                cell_off = np.zeros((GPC, NWIN), np.int64)
                cell_off.reshape(-1)[1:] = np.cumsum(T.reshape(-1))[:-1]
                # per-group tile counts/offsets (slot space, (g,w,t) order)
                Tg = T.sum(axis=1)
                g_off = np.zeros(GPC + 1, np.int64)
                g_off[1:] = np.cumsum(Tg)

                ci = 0
                col = 0
                call_cols = []
                for (wq, nt) in call_specs:
                    call_cols.append(col)
                    col += nt * 8
                bi = 0
                for (b0, b1) in batches:
                    vals = {}
                    voff = {}
                    for wq in range(NWIN):
                        nt = call_specs[bi * NWIN + wq][1]
                        if nt == 0:
                            vals[wq] = None
                            continue
                        it = mpool.tile([P, nt * 8], dt.int16, tag="gidx")
                        c0 = call_cols[bi * NWIN + wq]
                        nc.sync.dma_start(it[:], gidx[:, c0 : c0 + nt * 8])
                        vt = gpool.tile([P, nt * d_in], dt.float32, tag=f"vals{wq}")
                        nc.gpsimd.dma_gather_unrestricted(
                            out_ap=vt[:].rearrange("p (c e) -> p c e", c=nt),
                            in_ap=table_ap[wq * WIN :, :d_in],
                            idxs_ap=it[:],
                            num_idxs=nt * P,
                            num_idxs_reg=nt * P,
                            elem_size=d_in,
                            elem_step=64,
                            single_packet=False,
                        )
                        vals[wq] = vt
                        # in-call offset per group
                        o = 0
                        for g in range(b0, b1):
                            voff[(g, wq)] = o
                            o += int(T[g, wq])

                    for g in range(b0, b1):
                        TgT = int(Tg[g])
                        if TgT == 0:
                            continue
                        dtt = mpool.tile([P, TgT], dt.float32, tag="dst")
                        nc.sync.dma_start(
                            dtt[:], dstf[:, g_off[g] : g_off[g] + TgT]
                        )
                        oh = ohpool.tile([P, TgT * P], dt.float32, tag="oh")
                        nc.vector.tensor_tensor(
                            out=oh[:].rearrange("p (t c) -> p t c", t=TgT),
                            in0=dtt[:].rearrange("p (t o) -> p t o", o=1)
                            .to_broadcast([P, TgT, P]),
                            in1=iota_f[:].rearrange("p (t c) -> p t c", t=1)
                            .to_broadcast([P, TgT, P]),
                            op=mybir.AluOpType.is_equal,
                        )
                        if row_major:
                            agg = psA.tile([P, d_in], dt.float32, tag="agg")
                        else:
                            agg = psA.tile([d_in, P], dt.float32, tag="agg")
                        ti = 0
                        for wq in range(NWIN):
                            for t in range(int(T[g, wq])):
                                vcol = (voff[(g, wq)] + t) * d_in
                                if row_major:
                                    nc.tensor.matmul(
                                        out=agg[:],
                                        lhsT=oh[:, ti * P : (ti + 1) * P],
                                        rhs=vals[wq][:, vcol : vcol + d_in],
                                        start=(ti == 0),
                                        stop=(ti == TgT - 1),
                                    )
                                else:
                                    nc.tensor.matmul(
                                        out=agg[:],
                                        lhsT=vals[wq][:, vcol : vcol + d_in],
                                        rhs=oh[:, ti * P : (ti + 1) * P],
                                        start=(ti == 0),
                                        stop=(ti == TgT - 1),
                                    )
                                ti += 1
                        epilogue(g, agg)
                    bi += 1

            # ---------- epilogues ----------
            def conv_epilogue(l, d_in, wslice, bias_t, bslice, inv_col, relu,
                              write_feat, write_ag, agidx, feat_tile):
                def ep(g, agg):
                    aggs = epool.tile([d_in, P], dt.float32, tag="aggs")
                    nc.vector.tensor_copy(aggs[:], agg[:])
                    h = psB.tile([P, D], dt.float32, tag="mm")
                    nc.tensor.matmul(out=h[:], lhsT=aggs[:], rhs=wslice,
                                     start=True, stop=True)
                    hs = epool.tile([P, D], dt.float32, tag="hs")
                    nc.vector.tensor_tensor(
                        out=hs[:], in0=h[:],
                        in1=inv_loc[:, l * GPC + g : l * GPC + g + 1].to_broadcast([P, D]),
                        op=mybir.AluOpType.mult,
                    )
                    nc.vector.tensor_add(hs[:], hs[:], bias_t[:, bslice])
                    if relu:
                        nc.scalar.activation(hs[:], hs[:], mybir.ActivationFunctionType.Relu)
                        hg = epool.tile([P, D], dt.float32, tag="hg")
                        nc.vector.tensor_tensor(
                            out=hg[:], in0=hs[:],
                            in1=inv_loc[:, l * GPC + g : l * GPC + g + 1].to_broadcast([P, D]),
                            op=mybir.AluOpType.mult,
                        )
                        outv = hg
                    else:
                        outv = hs
                    if write_feat is not None:
                        nc.sync.dma_start(write_feat[g * P : (g + 1) * P, :], outv[:])
                    if write_ag is not None:
                        nc.sync.dma_start(
                            write_ag[g * P : (g + 1) * P, :D], outv[:]
                        )
                return ep

            def msg_epilogue(i, write_last_dram, write_ag):
                def ep(g, agg):
                    fb = epool.tile([P, D], dt.float32, tag="fb")
                    nc.sync.dma_start(fb[:], feat_dram[g * P : (g + 1) * P, :])
                    ls = epool.tile([P, D], dt.float32, tag="ls")
                    nc.vector.tensor_add(ls[:], agg[:], fb[:])
                    nc.vector.tensor_tensor(
                        out=ls[:], in0=ls[:], in1=invdeg[:, i * D : (i + 1) * D],
                        op=mybir.AluOpType.mult,
                    )
                    if write_last_dram is not None:
                        nc.sync.dma_start(
                            write_last_dram[g * P : (g + 1) * P, :], ls[:]
                        )
                    if write_ag is not None:
                        nc.sync.dma_start(write_ag[g * P : (g + 1) * P, :D], ls[:])
                return ep

            def allgather(idx):
                nc.gpsimd.collective_compute(
                    "AllGather",
                    mybir.AluOpType.bypass,
                    ins=[ag_in[idx][:]],
                    outs=[ag_out[idx][:]],
                    replica_groups=[core_ids],
                )

            # ---------- schedule ----------
            # layer 0
            run_round(0, g1tab[0][:], F,
                      conv_epilogue(0, F, w1_t[:, 0:H], b1_t, slice(0, H),
                                    None, True, None, ag_in[0][:], None, None))
            allgather(0)
            run_round(1, ag_out[0][:], D,
                      conv_epilogue(0, D, w2_t[:, 0:D], b2_t, slice(0, D),
                                    None, False, None, ag_in[4][:], None, None))
            allgather(4)  # last_0
            for ell in range(1, L):
                # conv1_ell
                run_round(2 * ell, g1tab[ell][:], F,
                          conv_epilogue(ell, F, w1_t[:, ell * H : (ell + 1) * H],
                                        b1_t, slice(ell * H, (ell + 1) * H),
                                        None, True, None, ag_in[ell][:], None, None))
                allgather(ell)
                # conv2_ell -> featbuf
                run_round(2 * ell + 1, ag_out[ell][:], D,
                          conv_epilogue(ell, D, w2_t[:, ell * D : (ell + 1) * D],
                                        b2_t, slice(ell * D, (ell + 1) * D),
                                        None, False, feat_dram, None, None, None))
                # msg_ell (round index 8 + ell-1), gathers last_{ell-1}
                last_tab = ag_out[4 + ell - 1]
                if ell < 3:
                    ep = msg_epilogue(ell, None, ag_in[4 + ell][:])
                else:
                    ep = msg_epilogue(ell, last3_dram, None)
                run_round(8 + ell - 1, last_tab[:], D, ep, row_major=True)
                if ell < 3:
                    allgather(4 + ell)

            # ---------- MLP ----------
            for g in range(GPC):
                rows = min(P, NPC - g * P)
                if rows <= 0:
                    break
                l3b = epool.tile([P, D], dt.float32, tag="l3b")
                nc.sync.dma_start(l3b[:], last3_dram[g * P : (g + 1) * P, :])
                l3T_ps = psC.tile([D, P], dt.float32, tag="mm2")
                nc.tensor.transpose(
                    out=l3T_ps[:], in_=l3b[:], identity=ident[:],
                )
                l3T = epool.tile([D, P], dt.float32, tag="l3Ts")
                nc.vector.tensor_copy(l3T[:], l3T_ps[:])
                h1T_ps = psB.tile([LH, P], dt.float32, tag="mm")
                nc.tensor.matmul(out=h1T_ps[:], lhsT=l1w_t[:], rhs=l3T[:],
                                 start=True, stop=True)
                h1T = epool.tile([LH, P], dt.float32, tag="h1Ts")
                nc.scalar.activation(h1T[:], h1T_ps[:],
                                     mybir.ActivationFunctionType.Relu,
                                     bias=l1b_t[:])
                o_ps = psC.tile([P, D], dt.float32, tag="mm2")
                nc.tensor.matmul(out=o_ps[:], lhsT=h1T[:], rhs=l2w_t[:],
                                 start=True, stop=True)
                ot = epool.tile([P, D], dt.float32, tag="ot")
                nc.vector.tensor_add(ot[:], o_ps[:], l2b_t[:])
                nc.scalar.activation(ot[:], ot[:], mybir.ActivationFunctionType.Relu)
                nc.sync.dma_start(out[g * P : g * P + rows, :], ot[:rows, :])

    nc.compile()
    return nc


# ---------------------------------------------------------------- runner

_CACHE = {}


def kernel(**inputs):
    import jax
    import concourse.mybir as mybir
    from jax.sharding import Mesh, PartitionSpec
    from jax.experimental.shard_map import shard_map
    from concourse import bass2jax
    from concourse.bass2jax import _bass_exec_p, install_neuronx_cc_hook

    rounds, consts, cnt_loc_t = preprocess(**inputs)
    nc = build_program(rounds, consts)

    install_neuronx_cc_hook()
    partition_name = nc.partition_id_tensor.name if nc.partition_id_tensor else None
    in_names, out_names, out_avals, zero_outs = [], [], [], []
    for alloc in nc.m.functions[0].allocations:
        if not isinstance(alloc, mybir.MemoryLocationSet):
            continue
        name = alloc.memorylocations[0].name
        if alloc.kind == "ExternalInput":
            if name != partition_name:
                in_names.append(name)
        elif alloc.kind == "ExternalOutput":
            shape = tuple(alloc.tensor_shape)
            dtype = mybir.dt.np(alloc.dtype)
            out_names.append(name)
            out_avals.append(jax.core.ShapedArray(shape, dtype))
            zero_outs.append(np.zeros(shape, dtype))

    # per-core input maps
    in_maps = []
    for c in range(NC):
        m = dict(consts)
        m["cnt_loc"] = cnt_loc_t[c]
        for r, (_, _, _, per_core) in enumerate(rounds):
            m[f"gidx{r}"] = per_core[c]["gidx"]
            m[f"dstf{r}"] = per_core[c]["dstf"]
        in_maps.append(m)

    all_in_names = list(in_names) + list(out_names)
    if partition_name is not None:
        all_in_names.append(partition_name)

    def _body(*args):
        operands = list(args)
        if partition_name is not None:
            operands.append(bass2jax.partition_id_tensor())
        outs = _bass_exec_p.bind(
            *operands,
            out_avals=tuple(out_avals),
            in_names=tuple(all_in_names),
            out_names=tuple(out_names),
            lowering_input_output_aliases=(),
            sim_require_finite=True,
            sim_require_nnan=True,
            nc=nc,
        )
        return tuple(outs)

    devices = jax.devices()[:NC]
    mesh = Mesh(np.asarray(devices), ("core",))
    n_params = len(in_names)
    in_specs = (PartitionSpec("core"),) * (n_params + len(out_names))
    out_specs = (PartitionSpec("core"),) * len(out_names)
    fn = jax.jit(
        shard_map(_body, mesh=mesh, in_specs=in_specs, out_specs=out_specs,
                  check_rep=False),
        keep_unused=True,
    )
    sh = jax.sharding.NamedSharding(mesh, PartitionSpec("core"))
    concat_in = [
        jax.device_put(
            np.concatenate([np.asarray(in_maps[c][n]) for c in range(NC)], axis=0), sh
        )
        for n in in_names
    ]
    concat_zeros = [
        jax.device_put(np.zeros((NC * z.shape[0], *z.shape[1:]), z.dtype), sh)
        for z in zero_outs
    ]
    outs = fn(*concat_in, *concat_zeros)
    jax.block_until_ready(outs)
    oi = out_names.index("out")
    full = np.asarray(outs[oi]).reshape(NC, NPC, D)
    kernel._timing_handle = (fn, concat_in, concat_zeros)
    return full.reshape(N, D)


# revision 5
# speedup vs baseline: 1.0640x; 1.0640x over previous
"""Trainium2 Bass kernel for nn_DS_EmbeddingGenerator (4-layer GCN + inter-layer
message passing + MLP head), 8-core SPMD.

Strategy (graph/data parallel per sharding hint):
- Nodes partitioned in 8 contiguous dst-slices of 25000 (padded to 25088 = 196
  groups of 128). Each core aggregates the edges whose dst lands in its slice.
- GCN factorization: conv(h,W,b) = inv[dst] * (segsum_{edges+self}((inv*h)[src])) @ W + b
  so no per-edge coefficients are needed; self-loops are appended to the edge list.
- Per-edge gather of table rows via dma_gather (int16 idx, 7 static 32768-row
  src windows; 128B payload on 256B-stride tables via a patched elem_size).
- Segment-sum on chip: f32 one-hot (is_equal vs iota) + PE matmul accumulating
  into PSUM per 128-dst group; epilogue applies inv scaling, weights, bias, relu.
- Full tables (h1*inv per layer; last_i) are exchanged with AllGather collectives.
- Final MLP fused per group; output written per-core slice, concatenated on host.
"""
import inspect
import textwrap
import warnings

import numpy as np

warnings.filterwarnings("ignore")

L, N, E = 4, 200000, 3200000
F, H, D, LH = 20, 32, 32, 64
NC = 8
NPC = 25000            # real nodes per core
P = 128
GPC = 196              # groups per core (196*128 = 25088)
NPAD = GPC * P         # padded nodes per core
WIN = 32768
TABROWS = NC * NPAD    # AG table rows (200704)
NWIN = (TABROWS + WIN - 1) // WIN  # 7
NB = 6                 # groups per gather batch

_f32 = np.float32


def _patch_gather():
    import concourse.bass as bass

    if hasattr(bass.BassGpSimd, "dma_gather_unrestricted"):
        return
    src = inspect.getsource(bass.BassGpSimd.dma_gather)
    src = src.replace(
        "assert (\n            elem_size_bytes > 0 and elem_size_bytes % 256 == 0\n        )  # transpose restriction",
        "assert elem_size_bytes > 0",
    )
    ns = vars(bass).copy()
    exec(textwrap.dedent(src), ns)
    bass.BassGpSimd.dma_gather_unrestricted = ns["dma_gather"]


# ---------------------------------------------------------------- host preprocessing

def _prep_round(dst, src, tabidx):
    """Shard one aggregation round across cores.

    dst: global dst (int64) in [0, N); src unused except via tabidx;
    tabidx: table row for each edge (int64) in [0, table rows).
    Returns (T, per-core dicts with gidx(int16 wrapped+replicated), dstf(f32)).
    T[g][w] tile table uniform across cores.
    """
    core = dst // NPC
    dloc = dst - core * NPC
    grp = dloc >> 7
    gloc = dloc & 127
    w = tabidx >> 15
    lidx = tabidx - (w << 15)

    cnt = np.zeros((NC, GPC, NWIN), np.int64)
    key_cgw = (core * GPC + grp) * NWIN + w
    np.add.at(cnt.reshape(-1), key_cgw, 1)
    tiles = -(-cnt // P)          # ceil
    T = tiles.max(axis=0)         # [GPC, NWIN]
    # safety: every (g,w) cell capacity
    total_tiles = int(T.sum())

    # slot layout per core: order (g, w, tile, pos); per (c,g,w) edges then pads
    # build with a global sort by (core, g, w, lidx)
    order = np.lexsort((lidx, w, grp, core))
    core_s = core[order]
    grp_s = grp[order]
    w_s = w[order]
    lidx_s = lidx[order]
    gloc_s = gloc[order]

    # slot base for each (g, w): offset in tiles
    cell_tile_off = np.zeros((GPC, NWIN), np.int64)
    flat_T = T.reshape(-1)
    cell_tile_off.reshape(-1)[1:] = np.cumsum(flat_T)[:-1]

    # rank of each edge within its (c,g,w) cell
    k = (core_s * GPC + grp_s) * NWIN + w_s
    is_new = np.empty(len(k), bool)
    is_new[0] = True
    is_new[1:] = k[1:] != k[:-1]
    seg_start = np.where(is_new)[0]
    rank = np.arange(len(k)) - np.repeat(seg_start, np.diff(np.append(seg_start, len(k))))

    slot = cell_tile_off.reshape(-1)[grp_s * NWIN + w_s] * P + rank

    per_core = []
    nslots = total_tiles * P
    for c in range(NC):
        m = core_s == c
        gi = np.zeros(nslots, np.int16)          # pad idx 0 (valid, zeroed by onehot)
        df = np.full(nslots, -1.0, _f32)          # pad dst -> one-hot all-zero
        gi[slot[m]] = lidx_s[m].astype(np.int16)
        df[slot[m]] = gloc_s[m].astype(_f32)
        per_core.append((gi, df))

    # gather-call structure: per (batch of NB groups, w): tiles
    batches = [(b0, min(b0 + NB, GPC)) for b0 in range(0, GPC, NB)]
    calls = []                     # (w, tile_start, ntiles) in slot space? need per-(g,w) runs
    # slot space is ordered g-major then w: a (batch,w) call covers non-contiguous
    # per-g runs -> instead make calls per (g,w) contiguous? Reorganize: we emit
    # calls per (batch, w) as the union of per-g cell ranges which ARE contiguous
    # in slot space only per (g,w). To keep one DMA per call, reorder slots:
    # order (w-major within batch). Simpler: calls per (g, w) are contiguous; we
    # bundle consecutive groups' (g,w) cells by issuing per (batch, w) a single
    # call over a REARRANGED index array built here: concat cells (g in batch).
    # We therefore build a separate call-ordered gidx array.
    call_specs = []                # (ntiles,) per call in emission order
    gidx_call_parts = [[] for _ in range(NC)]
    for (b0, b1) in batches:
        for wq in range(NWIN):
            nt = int(T[b0:b1, wq].sum())
            call_specs.append((wq, nt))
            if nt == 0:
                continue
            for c in range(NC):
                gi = per_core[c][0]
                parts = []
                for g in range(b0, b1):
                    t0 = cell_tile_off[g, wq] * P
                    parts.append(gi[t0 : t0 + T[g, wq] * P])
                gidx_call_parts[c].append(np.concatenate(parts))

    out_cores = []
    for c in range(NC):
        flat = (
            np.concatenate(gidx_call_parts[c])
            if gidx_call_parts[c]
            else np.zeros(0, np.int16)
        )
        # wrap 16 + replicate to 128 partitions
        wrapped = np.ascontiguousarray(flat.reshape(-1, 16).T)
        rep = np.tile(wrapped, (8, 1))
        # dstf in (g, w, t) order == slot order
        out_cores.append({"gidx": rep, "dstf": per_core[c][1].reshape(-1, P).T.copy()})
    return T, call_specs, batches, out_cores


def _self_loops():
    n = np.arange(N, dtype=np.int64)
    return n


def preprocess(xs, edge_index, layer_edge_index, deg_scalars,
               gcn_w1, gcn_b1, gcn_w2, gcn_b2, lin1_w, lin1_b, lin2_w, lin2_b):
    xs = np.asarray(xs, _f32)
    edge_index = np.asarray(edge_index, np.int64)
    layer_edge_index = np.asarray(layer_edge_index, np.int64)

    n = _self_loops()
    tr = lambda v: NPAD * (v // NPC) + (v % NPC)   # node id -> AG-table row

    rounds = []        # list of (T, call_specs, batches, per_core_arrays)
    # conv1/conv2 per layer
    for ell in range(L):
        d = edge_index[ell, 0]
        s = edge_index[ell, 1]
        dall = np.concatenate([d, n])
        sall = np.concatenate([s, n])
        rounds.append(_prep_round(dall, sall, sall))            # conv1: g1 natural rows
        rounds.append(_prep_round(dall, sall, tr(sall)))        # conv2: AG table rows
    for i in range(1, L):
        recv = layer_edge_index[i, 0]
        s = layer_edge_index[i, 1]
        rounds.append(_prep_round(recv, s, tr(s)))              # msg_i

    # counts for inv (in-degree per layer, global & per-core local)
    G_ALL = 1563  # ceil(N/128) with N padded to 200064
    NPADALL = G_ALL * P
    cnt_glob = np.zeros((L, NPADALL), _f32)
    for ell in range(L):
        cnt_glob[ell, :N] = np.bincount(edge_index[ell, 0], minlength=N).astype(_f32)
    cnt_glob_t = cnt_glob.reshape(L, G_ALL, P).transpose(0, 2, 1).copy()  # [L,128,1563]

    cnt_loc = np.zeros((NC, L, NPAD), _f32)
    for c in range(NC):
        cnt_loc[c, :, :NPC] = cnt_glob[:, c * NPC : (c + 1) * NPC]
    cnt_loc_t = cnt_loc.reshape(NC, L, GPC, P).transpose(0, 1, 3, 2).copy()  # [c][L,128,196]

    # xs tiled [L, 128, 1563*F]
    xs_pad = np.zeros((L, NPADALL, F), _f32)
    xs_pad[:, :N] = xs
    xs_t = xs_pad.reshape(L, G_ALL, P, F).transpose(0, 2, 1, 3).reshape(L, P, G_ALL * F).copy()

    consts = {
        "xs_t": xs_t,
        "cnt_glob": cnt_glob_t,
        "w1": np.asarray(gcn_w1, _f32),                     # [L,20,32]
        "b1_rep": np.tile(np.asarray(gcn_b1, _f32)[:, None, :], (1, P, 1)),  # [L,128,32]
        "w2": np.asarray(gcn_w2, _f32),                     # [L,32,32]
        "b2_rep": np.tile(np.asarray(gcn_b2, _f32)[:, None, :], (1, P, 1)),
        "lin1_w": np.asarray(lin1_w, _f32),                 # [32,64]
        "lin1_b_col": np.asarray(lin1_b, _f32)[:, None],    # [64,1]
        "lin2_w": np.asarray(lin2_w, _f32),                 # [64,32]
        "lin2_b_rep": np.tile(np.asarray(lin2_b, _f32)[None, :], (P, 1)),  # [128,32]
        "deg_rep": np.tile(np.asarray(deg_scalars, _f32)[:, None, None], (1, P, D)),  # [L,128,32]
    }
    return rounds, consts, cnt_loc_t


# ---------------------------------------------------------------- device program

def build_program(rounds, consts):
    import concourse.bass as bass
    import concourse.mybir as mybir
    import concourse.tile as tile
    from concourse import bacc
    from concourse.masks import make_identity

    _patch_gather()
    G_ALL = 1563

    nc = bacc.Bacc("TRN2", target_bir_lowering=False, debug=False, num_devices=NC)
    dt = mybir.dt

    # ---- I/O ----
    xs_t = nc.dram_tensor("xs_t", [L, P, G_ALL * F], dt.float32, kind="ExternalInput")
    cnt_glob = nc.dram_tensor("cnt_glob", [L, P, G_ALL], dt.float32, kind="ExternalInput")
    cnt_loc = nc.dram_tensor("cnt_loc", [L, P, GPC], dt.float32, kind="ExternalInput")
    w1_in = nc.dram_tensor("w1", [L, F, H], dt.float32, kind="ExternalInput")
    b1_in = nc.dram_tensor("b1_rep", [L, P, H], dt.float32, kind="ExternalInput")
    w2_in = nc.dram_tensor("w2", [L, H, D], dt.float32, kind="ExternalInput")
    b2_in = nc.dram_tensor("b2_rep", [L, P, D], dt.float32, kind="ExternalInput")
    l1w_in = nc.dram_tensor("lin1_w", [D, LH], dt.float32, kind="ExternalInput")
    l1b_in = nc.dram_tensor("lin1_b_col", [LH, 1], dt.float32, kind="ExternalInput")
    l2w_in = nc.dram_tensor("lin2_w", [LH, D], dt.float32, kind="ExternalInput")
    l2b_in = nc.dram_tensor("lin2_b_rep", [P, D], dt.float32, kind="ExternalInput")
    deg_in = nc.dram_tensor("deg_rep", [L, P, D], dt.float32, kind="ExternalInput")

    g_ins, d_ins = [], []
    for r, (T, call_specs, batches, per_core) in enumerate(rounds):
        gshape = per_core[0]["gidx"].shape
        dshape = per_core[0]["dstf"].shape
        g_ins.append(nc.dram_tensor(f"gidx{r}", list(gshape), dt.int16, kind="ExternalInput"))
        d_ins.append(nc.dram_tensor(f"dstf{r}", list(dshape), dt.float32, kind="ExternalInput"))

    out = nc.dram_tensor("out", [NPC, D], dt.float32, kind="ExternalOutput")

    # internal DRAM
    g1tab = [nc.dram_tensor(f"g1tab{l}", [G_ALL * P, 64], dt.float32) for l in range(L)]
    feat_dram = nc.dram_tensor("feat_dram", [NPAD, D], dt.float32)
    last3_dram = nc.dram_tensor("last3_dram", [NPAD, D], dt.float32)
    ag_in = [nc.dram_tensor(f"agin{i}", [NPAD, 64], dt.float32) for i in range(7)]
    ag_out = [
        nc.dram_tensor(f"agout{i}", [TABROWS, 64], dt.float32, addr_space="Shared")
        for i in range(7)
    ]
    # AG index: 0..3 = g2 tables layers 0..3 ; 4..6 = last_0..last_2

    core_ids = list(range(NC))

    with tile.TileContext(nc) as tc:
        with (
            tc.tile_pool(name="const", bufs=1) as cpool,
            tc.tile_pool(name="g1p", bufs=3) as g1pool,
            tc.tile_pool(name="gather", bufs=3) as gpool,
            tc.tile_pool(name="meta", bufs=4) as mpool,
            tc.tile_pool(name="oh", bufs=3) as ohpool,
            tc.tile_pool(name="epi", bufs=4) as epool,
            tc.tile_pool(name="prol", bufs=2) as ppool,
            tc.tile_pool(name="psA", bufs=3, space="PSUM") as psA,
            tc.tile_pool(name="psB", bufs=2, space="PSUM") as psB,
            tc.tile_pool(name="psC", bufs=2, space="PSUM") as psC,
        ):
            # ---------- constants ----------
            iota_i = cpool.tile([P, P], dt.int32)
            nc.gpsimd.iota(iota_i[:], pattern=[[1, P]], base=0, channel_multiplier=0)
            iota_f = cpool.tile([P, P], dt.float32)
            nc.vector.tensor_copy(iota_f[:], iota_i[:])
            ident = cpool.tile([P, P], dt.float32)
            make_identity(nc, ident[:])
            ones_col = cpool.tile([P, 1], dt.float32)
            nc.gpsimd.memset(ones_col[:], 1.0)

            w1_t = cpool.tile([F, L * H], dt.float32)
            for l in range(L):
                nc.sync.dma_start(w1_t[:, l * H : (l + 1) * H], w1_in[l])
            w2_t = cpool.tile([H, L * D], dt.float32)
            for l in range(L):
                nc.sync.dma_start(w2_t[:, l * D : (l + 1) * D], w2_in[l])
            b1_t = cpool.tile([P, L * H], dt.float32)
            for l in range(L):
                nc.sync.dma_start(b1_t[:, l * H : (l + 1) * H], b1_in[l])
            b2_t = cpool.tile([P, L * D], dt.float32)
            for l in range(L):
                nc.sync.dma_start(b2_t[:, l * D : (l + 1) * D], b2_in[l])
            l1w_t = cpool.tile([D, LH], dt.float32)
            nc.sync.dma_start(l1w_t[:], l1w_in[:])
            l1b_t = cpool.tile([LH, 1], dt.float32)
            nc.sync.dma_start(l1b_t[:], l1b_in[:])
            l2w_t = cpool.tile([LH, D], dt.float32)
            nc.sync.dma_start(l2w_t[:], l2w_in[:])
            l2b_t = cpool.tile([P, D], dt.float32)
            nc.sync.dma_start(l2b_t[:], l2b_in[:])
            degr = cpool.tile([P, L * D], dt.float32)
            for l in range(L):
                nc.sync.dma_start(degr[:, l * D : (l + 1) * D], deg_in[l])
            invdeg = cpool.tile([P, L * D], dt.float32)
            nc.vector.reciprocal(invdeg[:], degr[:])

            # inv tables
            inv_glob = cpool.tile([P, L * G_ALL], dt.float32)
            inv_loc = cpool.tile([P, L * GPC], dt.float32)
            for l in range(L):
                cg = ppool.tile([P, G_ALL], dt.float32, tag="cg")
                nc.sync.dma_start(cg[:], cnt_glob[l])
                nc.vector.tensor_scalar_add(cg[:], cg[:], 1.0)
                nc.vector.reciprocal(cg[:], cg[:])
                nc.scalar.activation(
                    inv_glob[:, l * G_ALL : (l + 1) * G_ALL], cg[:],
                    mybir.ActivationFunctionType.Sqrt,
                )
                cl = ppool.tile([P, GPC], dt.float32, tag="cl")
                nc.sync.dma_start(cl[:], cnt_loc[l])
                nc.vector.tensor_scalar_add(cl[:], cl[:], 1.0)
                nc.vector.reciprocal(cl[:], cl[:])
                nc.scalar.activation(
                    inv_loc[:, l * GPC : (l + 1) * GPC], cl[:],
                    mybir.ActivationFunctionType.Sqrt,
                )


            # ---------- g1 table builds ----------
            XB = 32
            for l in range(L):
                g1ap = g1tab[l][:].rearrange("(g p) e -> p g e", p=P)
                for G0 in range(0, G_ALL, XB):
                    B = min(XB, G_ALL - G0)
                    xt = g1pool.tile([P, XB * F], dt.float32, tag="xt")
                    nc.sync.dma_start(
                        xt[:, : B * F], xs_t[l][:, G0 * F : (G0 + B) * F]
                    )
                    sc = g1pool.tile([P, XB * F], dt.float32, tag="sc")
                    nc.vector.tensor_tensor(
                        out=sc[:, : B * F].rearrange("p (b f) -> p b f", b=B),
                        in0=xt[:, : B * F].rearrange("p (b f) -> p b f", b=B),
                        in1=inv_glob[:, l * G_ALL + G0 : l * G_ALL + G0 + B]
                        .rearrange("p (b o) -> p b o", o=1)
                        .to_broadcast([P, B, F]),
                        op=mybir.AluOpType.mult,
                    )
                    nc.sync.dma_start(
                        g1ap[:, G0 : G0 + B, :F],
                        sc[:, : B * F].rearrange("p (b f) -> p b f", b=B),
                    )

            # ---------- round runner ----------
            def run_round(r, table_ap, d_in, epilogue, row_major=False):
                T, call_specs, batches, _ = rounds[r]
                gidx = g_ins[r]
                dstf = d_ins[r]
                cell_off = np.zeros((GPC, NWIN), np.int64)
                cell_off.reshape(-1)[1:] = np.cumsum(T.reshape(-1))[:-1]
                # per-group tile counts/offsets (slot space, (g,w,t) order)
                Tg = T.sum(axis=1)
                g_off = np.zeros(GPC + 1, np.int64)
                g_off[1:] = np.cumsum(Tg)

                ci = 0
                col = 0
                call_cols = []
                for (wq, nt) in call_specs:
                    call_cols.append(col)
                    col += nt * 8
                bi = 0
                for (b0, b1) in batches:
                    vals = {}
                    voff = {}
                    for wq in range(NWIN):
                        nt = call_specs[bi * NWIN + wq][1]
                        if nt == 0:
                            vals[wq] = None
                            continue
                        it = mpool.tile([P, nt * 8], dt.int16, tag="gidx")
                        c0 = call_cols[bi * NWIN + wq]
                        nc.sync.dma_start(it[:], gidx[:, c0 : c0 + nt * 8])
                        vt = gpool.tile([P, nt * d_in], dt.float32, tag=f"vals{wq}")
                        nc.gpsimd.dma_gather_unrestricted(
                            out_ap=vt[:].rearrange("p (c e) -> p c e", c=nt),
                            in_ap=table_ap[wq * WIN :, :d_in],
                            idxs_ap=it[:],
                            num_idxs=nt * P,
                            num_idxs_reg=nt * P,
                            elem_size=d_in,
                            elem_step=64,
                            single_packet=False,
                        )
                        vals[wq] = vt
                        # in-call offset per group
                        o = 0
                        for g in range(b0, b1):
                            voff[(g, wq)] = o
                            o += int(T[g, wq])

                    for g in range(b0, b1):
                        TgT = int(Tg[g])
                        if TgT == 0:
                            continue
                        dtt = mpool.tile([P, TgT], dt.float32, tag="dst")
                        nc.sync.dma_start(
                            dtt[:], dstf[:, g_off[g] : g_off[g] + TgT]
                        )
                        oh = ohpool.tile([P, TgT * P], dt.float32, tag="oh")
                        nc.vector.tensor_tensor(
                            out=oh[:].rearrange("p (t c) -> p t c", t=TgT),
                            in0=dtt[:].rearrange("p (t o) -> p t o", o=1)
                            .to_broadcast([P, TgT, P]),
                            in1=iota_f[:].rearrange("p (t c) -> p t c", t=1)
                            .to_broadcast([P, TgT, P]),
                            op=mybir.AluOpType.is_equal,
                        )
                        if row_major:
                            agg = psA.tile([P, d_in], dt.float32, tag="agg")
                        else:
                            agg = psA.tile([d_in, P], dt.float32, tag="agg")
                        ti = 0
                        for wq in range(NWIN):
                            for t in range(int(T[g, wq])):
                                vcol = (voff[(g, wq)] + t) * d_in
                                if row_major:
                                    nc.tensor.matmul(
                                        out=agg[:],
                                        lhsT=oh[:, ti * P : (ti + 1) * P],
                                        rhs=vals[wq][:, vcol : vcol + d_in],
                                        start=(ti == 0),
                                        stop=(ti == TgT - 1),
                                    )
                                else:
                                    nc.tensor.matmul(
                                        out=agg[:],
                                        lhsT=vals[wq][:, vcol : vcol + d_in],
                                        rhs=oh[:, ti * P : (ti + 1) * P],
                                        start=(ti == 0),
                                        stop=(ti == TgT - 1),
                                    )
                                ti += 1
                        epilogue(g, agg)
                    bi += 1

            # ---------- epilogues ----------
            def conv_epilogue(l, d_in, wslice, bias_t, bslice, inv_col, relu,
                              write_feat, write_ag, agidx, feat_tile):
                def ep(g, agg):
                    aggs = epool.tile([d_in, P], dt.float32, tag="aggs")
                    nc.vector.tensor_copy(aggs[:], agg[:])
                    h = psB.tile([P, D], dt.float32, tag="mm")
                    nc.tensor.matmul(out=h[:], lhsT=aggs[:], rhs=wslice,
                                     start=True, stop=True)
                    hs = epool.tile([P, D], dt.float32, tag="hs")
                    nc.vector.tensor_tensor(
                        out=hs[:], in0=h[:],
                        in1=inv_loc[:, l * GPC + g : l * GPC + g + 1].to_broadcast([P, D]),
                        op=mybir.AluOpType.mult,
                    )
                    nc.vector.tensor_add(hs[:], hs[:], bias_t[:, bslice])
                    if relu:
                        nc.scalar.activation(hs[:], hs[:], mybir.ActivationFunctionType.Relu)
                        hg = epool.tile([P, D], dt.float32, tag="hg")
                        nc.vector.tensor_tensor(
                            out=hg[:], in0=hs[:],
                            in1=inv_loc[:, l * GPC + g : l * GPC + g + 1].to_broadcast([P, D]),
                            op=mybir.AluOpType.mult,
                        )
                        outv = hg
                    else:
                        outv = hs
                    if write_feat is not None:
                        nc.sync.dma_start(write_feat[g * P : (g + 1) * P, :], outv[:])
                    if write_ag is not None:
                        nc.sync.dma_start(
                            write_ag[g * P : (g + 1) * P, :D], outv[:]
                        )
                return ep

            def msg_epilogue(i, write_last_dram, write_ag):
                def ep(g, agg):
                    fb = epool.tile([P, D], dt.float32, tag="fb")
                    nc.sync.dma_start(fb[:], feat_dram[g * P : (g + 1) * P, :])
                    ls = epool.tile([P, D], dt.float32, tag="ls")
                    nc.vector.tensor_add(ls[:], agg[:], fb[:])
                    nc.vector.tensor_tensor(
                        out=ls[:], in0=ls[:], in1=invdeg[:, i * D : (i + 1) * D],
                        op=mybir.AluOpType.mult,
                    )
                    if write_last_dram is not None:
                        nc.sync.dma_start(
                            write_last_dram[g * P : (g + 1) * P, :], ls[:]
                        )
                    if write_ag is not None:
                        nc.sync.dma_start(write_ag[g * P : (g + 1) * P, :D], ls[:])
                return ep

            def allgather(idx):
                nc.gpsimd.collective_compute(
                    "AllGather",
                    mybir.AluOpType.bypass,
                    ins=[ag_in[idx][:]],
                    outs=[ag_out[idx][:]],
                    replica_groups=[core_ids],
                )

            # ---------- schedule ----------
            # layer 0
            run_round(0, g1tab[0][:], F,
                      conv_epilogue(0, F, w1_t[:, 0:H], b1_t, slice(0, H),
                                    None, True, None, ag_in[0][:], None, None))
            allgather(0)
            run_round(1, ag_out[0][:], D,
                      conv_epilogue(0, D, w2_t[:, 0:D], b2_t, slice(0, D),
                                    None, False, None, ag_in[4][:], None, None))
            allgather(4)  # last_0
            for ell in range(1, L):
                # conv1_ell
                run_round(2 * ell, g1tab[ell][:], F,
                          conv_epilogue(ell, F, w1_t[:, ell * H : (ell + 1) * H],
                                        b1_t, slice(ell * H, (ell + 1) * H),
                                        None, True, None, ag_in[ell][:], None, None))
                allgather(ell)
                # conv2_ell -> featbuf
                run_round(2 * ell + 1, ag_out[ell][:], D,
                          conv_epilogue(ell, D, w2_t[:, ell * D : (ell + 1) * D],
                                        b2_t, slice(ell * D, (ell + 1) * D),
                                        None, False, feat_dram, None, None, None))
                # msg_ell (round index 8 + ell-1), gathers last_{ell-1}
                last_tab = ag_out[4 + ell - 1]
                if ell < 3:
                    ep = msg_epilogue(ell, None, ag_in[4 + ell][:])
                else:
                    ep = msg_epilogue(ell, last3_dram, None)
                run_round(8 + ell - 1, last_tab[:], D, ep, row_major=True)
                if ell < 3:
                    allgather(4 + ell)

            # ---------- MLP ----------
            for g in range(GPC):
                rows = min(P, NPC - g * P)
                if rows <= 0:
                    break
                l3b = epool.tile([P, D], dt.float32, tag="l3b")
                nc.sync.dma_start(l3b[:], last3_dram[g * P : (g + 1) * P, :])
                l3T_ps = psC.tile([D, P], dt.float32, tag="mm2")
                nc.tensor.transpose(
                    out=l3T_ps[:], in_=l3b[:], identity=ident[:],
                )
                l3T = epool.tile([D, P], dt.float32, tag="l3Ts")
                nc.vector.tensor_copy(l3T[:], l3T_ps[:])
                h1T_ps = psB.tile([LH, P], dt.float32, tag="mm")
                nc.tensor.matmul(out=h1T_ps[:], lhsT=l1w_t[:], rhs=l3T[:],
                                 start=True, stop=True)
                h1T = epool.tile([LH, P], dt.float32, tag="h1Ts")
                nc.scalar.activation(h1T[:], h1T_ps[:],
                                     mybir.ActivationFunctionType.Relu,
                                     bias=l1b_t[:])
                o_ps = psC.tile([P, D], dt.float32, tag="mm2")
                nc.tensor.matmul(out=o_ps[:], lhsT=h1T[:], rhs=l2w_t[:],
                                 start=True, stop=True)
                ot = epool.tile([P, D], dt.float32, tag="ot")
                nc.vector.tensor_add(ot[:], o_ps[:], l2b_t[:])
                nc.scalar.activation(ot[:], ot[:], mybir.ActivationFunctionType.Relu)
                nc.sync.dma_start(out[g * P : g * P + rows, :], ot[:rows, :])

    nc.compile()
    return nc


# ---------------------------------------------------------------- runner

_CACHE = {}


def kernel(**inputs):
    import jax
    import concourse.mybir as mybir
    from jax.sharding import Mesh, PartitionSpec
    from jax.experimental.shard_map import shard_map
    from concourse import bass2jax
    from concourse.bass2jax import _bass_exec_p, install_neuronx_cc_hook

    rounds, consts, cnt_loc_t = preprocess(**inputs)
    nc = build_program(rounds, consts)

    install_neuronx_cc_hook()
    partition_name = nc.partition_id_tensor.name if nc.partition_id_tensor else None
    in_names, out_names, out_avals, zero_outs = [], [], [], []
    for alloc in nc.m.functions[0].allocations:
        if not isinstance(alloc, mybir.MemoryLocationSet):
            continue
        name = alloc.memorylocations[0].name
        if alloc.kind == "ExternalInput":
            if name != partition_name:
                in_names.append(name)
        elif alloc.kind == "ExternalOutput":
            shape = tuple(alloc.tensor_shape)
            dtype = mybir.dt.np(alloc.dtype)
            out_names.append(name)
            out_avals.append(jax.core.ShapedArray(shape, dtype))
            zero_outs.append(np.zeros(shape, dtype))

    # per-core input maps
    in_maps = []
    for c in range(NC):
        m = dict(consts)
        m["cnt_loc"] = cnt_loc_t[c]
        for r, (_, _, _, per_core) in enumerate(rounds):
            m[f"gidx{r}"] = per_core[c]["gidx"]
            m[f"dstf{r}"] = per_core[c]["dstf"]
        in_maps.append(m)

    all_in_names = list(in_names) + list(out_names)
    if partition_name is not None:
        all_in_names.append(partition_name)

    def _body(*args):
        operands = list(args)
        if partition_name is not None:
            operands.append(bass2jax.partition_id_tensor())
        outs = _bass_exec_p.bind(
            *operands,
            out_avals=tuple(out_avals),
            in_names=tuple(all_in_names),
            out_names=tuple(out_names),
            lowering_input_output_aliases=(),
            sim_require_finite=True,
            sim_require_nnan=True,
            nc=nc,
        )
        return tuple(outs)

    devices = jax.devices()[:NC]
    mesh = Mesh(np.asarray(devices), ("core",))
    n_params = len(in_names)
    in_specs = (PartitionSpec("core"),) * (n_params + len(out_names))
    out_specs = (PartitionSpec("core"),) * len(out_names)
    fn = jax.jit(
        shard_map(_body, mesh=mesh, in_specs=in_specs, out_specs=out_specs,
                  check_rep=False),
        keep_unused=True,
    )
    sh = jax.sharding.NamedSharding(mesh, PartitionSpec("core"))
    concat_in = [
        jax.device_put(
            np.concatenate([np.asarray(in_maps[c][n]) for c in range(NC)], axis=0), sh
        )
        for n in in_names
    ]
    concat_zeros = [
        jax.device_put(np.zeros((NC * z.shape[0], *z.shape[1:]), z.dtype), sh)
        for z in zero_outs
    ]
    outs = fn(*concat_in, *concat_zeros)
    jax.block_until_ready(outs)
    oi = out_names.index("out")
    full = np.asarray(outs[oi]).reshape(NC, NPC, D)
    kernel._timing_handle = (fn, concat_in, concat_zeros)
    return full.reshape(N, D)
